# revision 18
# baseline (speedup 1.0000x reference)
"""AttFlowLayer TRN2 Bass kernel — data-parallel over batch across 8 NeuronCores.

Math (per batch element b):
  S[i,j]  = cw[i] + qw[j] + sum_d ctx[i,d]*wm[d]*q[j,d]      (qw[j] cancels in
            the softmax over i, so it is dropped entirely)
  P[i,j]  = exp(S_mm[i,j] + cw[i])          (no max-subtraction needed: |S|<~8)
  denom[j]= sum_i P[i,j]        r[j] = qmask[j]/denom[j]
  P'[i,j] = P[i,j]*r[j]
  H[j,d]  = sum_i P'[i,j]*ctx[i,d]
  colsum[i]= sum_j P'[i,j]      G = [ctx | ctx*colsum[:,None]]

Matmuls run in float32r (TF32-like, 1 cycle/row).  HW rounds raw fp32 input
bits internally, so fp32 data is loaded into float32r-typed tiles via bitcast
DMAs; exact-fp32 paths (G's context copy) read the same tiles bitcast back.
"""
import os
import numpy as np

STAGE = int(os.environ.get("K_STAGE", "9"))

B, LC, LQ, D = 16, 2048, 1024, 512
N_CORES = 8
B_LOC = B // N_CORES          # 2 batch elements per core
NT_I = LC // 128              # 16 i-tiles
NT_J = LQ // 128              # 8 j-tiles
KC = D // 128                 # 4 contraction chunks
JB = LQ // 512                # 2 j-blocks of 512

_BUILT = {}


def _split_excess_waits(nc, mybir):
    """walrus allows 1 sync-wait per instruction (EventSemaphore: 2); move
    the excess onto EventSemaphore instructions inserted just before."""
    engines = (
        mybir.EngineType.PE,
        mybir.EngineType.DVE,
        mybir.EngineType.Activation,
        mybir.EngineType.Pool,
        mybir.EngineType.SP,
    )
    n_fixed = 0
    for fn in nc.m.functions:
        for blk in fn.blocks:
            insts = list(blk.instructions)
            out = []
            changed = False
            for inst in insts:
                si = inst.sync_info
                cap = 2 if isinstance(inst, mybir.InstEventSemaphore) else 1
                if (
                    inst.engine in engines
                    and si is not None
                    and si.on_wait is not None
                    and len(si.on_wait) > cap
                ):
                    waits = list(si.on_wait)
                    keep = waits[-cap:]
                    excess = waits[:-cap]
                    for i in range(0, len(excess), 2):
                        ev = mybir.InstEventSemaphore(
                            name=nc.get_next_instruction_name(), ins=[], outs=[]
                        )
                        ev.engine = inst.engine
                        ev.debug = inst.debug
                        ev.sync_info = mybir.SyncInfo(
                            on_wait=excess[i : i + 2], on_update=[]
                        )
                        nc.register_instruction(ev)
                        out.append(ev)
                    inst.sync_info = mybir.SyncInfo(
                        on_wait=keep, on_update=list(si.on_update or [])
                    )
                    n_fixed += 1
                    changed = True
                out.append(inst)
            if changed:
                blk.instructions = out
    return n_fixed


def _build():
    if "nc" in _BUILT:
        return _BUILT["nc"]

    from contextlib import ExitStack
    import concourse.bass as bass
    import concourse.mybir as mybir
    import concourse.tile as tile

    FP = mybir.dt.float32
    FR = mybir.dt.float32r
    ALU = mybir.AluOpType
    AF = mybir.ActivationFunctionType
    AX = mybir.AxisListType

    nc = bass.Bass()
    ctx_d = nc.declare_dram_parameter("context", [B_LOC, LC, D], FP, isOutput=False)
    q_d = nc.declare_dram_parameter("query", [LQ, D], FP, isOutput=False)
    w_d = nc.declare_dram_parameter("w", [1, 3 * D], FP, isOutput=False)
    ident_d = nc.declare_dram_parameter("ident", [128, 128], FP, isOutput=False)
    ones_d = nc.declare_dram_parameter("ones", [128, 1], FP, isOutput=False)
    G_d = nc.declare_dram_parameter("G", [B_LOC, LC, 2 * D], FP, isOutput=True)
    H_d = nc.declare_dram_parameter("H", [B_LOC, LQ, D], FP, isOutput=True)

    with tile.TileContext(nc) as tc, ExitStack() as ctx:
        singles = ctx.enter_context(tc.tile_pool(name="singles", bufs=1))
        ctxp = ctx.enter_context(tc.tile_pool(name="ctxp", bufs=2))
        pp = ctx.enter_context(tc.tile_pool(name="pp", bufs=1))
        stage = ctx.enter_context(tc.tile_pool(name="stage", bufs=2))
        small = ctx.enter_context(tc.tile_pool(name="small", bufs=2))
        ps_t = ctx.enter_context(tc.tile_pool(name="ps_t", bufs=2, space="PSUM"))
        ps_s = ctx.enter_context(tc.tile_pool(name="ps_s", bufs=3, space="PSUM"))
        ps_m = ctx.enter_context(tc.tile_pool(name="ps_m", bufs=2, space="PSUM"))
        ps_h = ctx.enter_context(tc.tile_pool(name="ps_h", bufs=1, space="PSUM"))

        # ---- constants ----
        ident = singles.tile([128, 128], FP)
        nc.sync.dma_start(out=ident, in_=ident_d[:, :])
        ones_col_fr = singles.tile([128, 1], FR)
        nc.sync.dma_start(out=ones_col_fr, in_=ones_d[:, :].bitcast(FR))
        ones_row_fp = singles.tile([1, 128], FP)
        nc.sync.dma_start(out=ones_row_fp, in_=ones_d.rearrange("p one -> one p"))
        # wm / wc as [128, KC] column layouts (element c*128+p -> [p, c])
        wm_col = singles.tile([128, KC], FP)
        nc.sync.dma_start(
            out=wm_col, in_=w_d[0:1, 2 * D : 3 * D].rearrange("one (c p) -> p (one c)", p=128)
        )
        wc_col = singles.tile([128, KC], FP)
        nc.sync.dma_start(
            out=wc_col, in_=w_d[0:1, 0:D].rearrange("one (c p) -> p (one c)", p=128)
        )

        # ---- query setup ----
        q_nat = ctxp.tile([128, NT_J, D], FP, tag="ctx")
        nc.sync.dma_start(out=q_nat, in_=q_d.rearrange("(t p) d -> p t d", p=128))

        # qsum[j] = sum_d q[j, d]  -> [128, NT_J] (j on partitions)
        qsum_col = singles.tile([128, NT_J], FP)
        for jt in range(NT_J):
            nc.vector.reduce_sum(
                out=qsum_col[:, jt : jt + 1], in_=q_nat[:, jt, :], axis=AX.X
            )
        # transpose -> [NT_J, 128] rows; qmask = (qsum != 0)
        ps_qm = ps_m.tile([NT_J, 128], FP, tag="m")
        nc.tensor.transpose(ps_qm, qsum_col, ident)
        qmask_rows = singles.tile([NT_J, 128], FP)
        nc.vector.tensor_scalar(
            out=qmask_rows, in0=ps_qm, scalar1=0.0, scalar2=None, op0=ALU.not_equal
        )
        qmask_row = singles.tile([1, LQ], FP)
        nc.sync.dma_start(
            out=qmask_row.rearrange("one (t j) -> one t j", j=128), in_=qmask_rows
        )

        # qmT[d, j] = wm[d] * q[j, d]^T  (+ extra col 1024 = wc[d])  as FR
        qmT = singles.tile([128, KC, LQ + 8], FR)
        for dc in range(KC):
            for jh in range(2):
                ps_q = ps_t.tile([128, 512], FP, tag="t")
                for jq in range(4):
                    jt = jh * 4 + jq
                    nc.tensor.transpose(
                        ps_q[:, jq * 128 : (jq + 1) * 128],
                        q_nat[:, jt, dc * 128 : (dc + 1) * 128],
                        ident,
                    )
                nc.scalar.mul(
                    out=qmT[:, dc, jh * 512 : (jh + 1) * 512],
                    in_=ps_q,
                    mul=wm_col[:, dc : dc + 1],
                )
            nc.scalar.copy(out=qmT[:, dc, LQ : LQ + 1], in_=wc_col[:, dc : dc + 1])

        if STAGE == 0:
            # debug: dump qmT into G[0] rows 0..512 (4 chunks of 128 x 1024)
            for dc in range(KC):
                qd = stage.tile([128, LQ], FP, tag="h_sb", name=f"qd{dc}")
                nc.scalar.copy(out=qd, in_=qmT[:, dc, 0:LQ].bitcast(FP))
                nc.sync.dma_start(out=G_d[0, dc * 128 : (dc + 1) * 128, :], in_=qd)
        # ---- per-batch pipeline ----
        for b in range(B_LOC if STAGE > 0 else 0):
            ctx_fr = ctxp.tile([128, NT_I, D], FR, tag="ctx")
            for q4 in range(4):
                nc.sync.dma_start(
                    out=ctx_fr[:, q4 * 4 : (q4 + 1) * 4, :],
                    in_=ctx_d[b]
                    .rearrange("(t p) d -> p t d", p=128)[:, q4 * 4 : (q4 + 1) * 4, :]
                    .bitcast(FR),
                )
                # G left half = exact context copy (DRAM -> DRAM, untouched bits)
                nc.sync.dma_start(
                    out=G_d[b, q4 * 512 : (q4 + 1) * 512, 0:D],
                    in_=ctx_d[b, q4 * 512 : (q4 + 1) * 512, :],
                )

            P = pp.tile([128, NT_I, LQ], FR, tag="P")
            cw_sb = small.tile([128, NT_I], FP, tag="cw")

            for it in range(NT_I):
                # transpose ctx i-tile -> ctxT [d, i] chunks (one PSUM bank)
                ps_T = ps_t.tile([128, 512], FP, tag="t")
                for k in range(KC):
                    nc.tensor.transpose(
                        ps_T[:, k * 128 : (k + 1) * 128],
                        ctx_fr[:, it, k * 128 : (k + 1) * 128].bitcast(FP),
                        ident,
                    )
                ctxT_t = stage.tile([128, KC, 128], FR, tag="ctxT")
                nc.scalar.copy(out=ctxT_t.rearrange("p k i -> p (k i)"), in_=ps_T)

                # MM1: S tiles + cw column
                ps_S = [ps_s.tile([128, 512], FP, tag="s", name=f"ps_S{jb_}") for jb_ in range(JB)]
                ps_cw = ps_m.tile([128, 8], FP, tag="m")
                for k in range(KC):
                    nc.tensor.matmul(
                        ps_cw,
                        ctxT_t[:, k, :],
                        qmT[:, k, LQ : LQ + 8],
                        start=(k == 0),
                        stop=(k == KC - 1),
                    )
                for jb in range(JB):
                    for k in range(KC):
                        nc.tensor.matmul(
                            ps_S[jb],
                            ctxT_t[:, k, :],
                            qmT[:, k, jb * 512 : (jb + 1) * 512],
                            start=(k == 0),
                            stop=(k == KC - 1),
                        )
                nc.vector.tensor_copy(cw_sb[:, it : it + 1], ps_cw[:, 0:1])
                for jb in range(JB):
                    nc.scalar.activation(
                        P[:, it, jb * 512 : (jb + 1) * 512],
                        ps_S[jb],
                        AF.Exp,
                        bias=cw_sb[:, it : it + 1],
                        scale=1.0,
                    )

            if STAGE < 2:
                # debug: dump P (post-exp) into G right half rows, cw into H
                for it in range(NT_I):
                    pdump = stage.tile([128, LQ], FP, tag="h_sb", name=f"pd{b}_{it}")
                    nc.scalar.copy(out=pdump, in_=P[:, it, :].bitcast(FP))
                    nc.sync.dma_start(
                        out=G_d[b, it * 128 : (it + 1) * 128, :], in_=pdump
                    )
                hdump = stage.tile([128, NT_I], FP, tag="gat", name=f"hd{b}")
                nc.vector.tensor_copy(hdump, cw_sb)
                nc.sync.dma_start(out=H_d[b, 0:128, 0:NT_I], in_=hdump)
                continue
            # denom[j] = sum_i P[i, j] via ones-matmul
            ps_den = [ps_m.tile([1, 512], FP, tag="m", name=f"ps_den{jb_}") for jb_ in range(JB)]
            for jb in range(JB):
                for ic in range(NT_I):
                    nc.tensor.matmul(
                        ps_den[jb],
                        ones_col_fr,
                        P[:, ic, jb * 512 : (jb + 1) * 512],
                        start=(ic == 0),
                        stop=(ic == NT_I - 1),
                    )
            # per j-half: r, broadcast, fold, then MM2 for that half's j-tiles
            for jb in range(JB):
                rinv = small.tile([1, 512], FP, tag="r", name=f"rinv{b}_{jb}")
                nc.vector.reciprocal(rinv, ps_den[jb])
                r_sb = small.tile([1, 512], FP, tag="r2", name=f"rsb{b}_{jb}")
                nc.vector.tensor_tensor(
                    out=r_sb, in0=rinv,
                    in1=qmask_row[:, jb * 512 : (jb + 1) * 512], op=ALU.mult,
                )
                ps_r = ps_m.tile([128, 512], FP, tag="m", name=f"psr{b}_{jb}")
                nc.tensor.matmul(ps_r, ones_row_fp, r_sb, start=True, stop=True)
                r128 = small.tile([128, 512], FP, tag="r128", name=f"r128_{b}_{jb}")
                nc.scalar.copy(out=r128, in_=ps_r)
                for it in range(NT_I):
                    nc.vector.tensor_tensor(
                        out=P[:, it, jb * 512 : (jb + 1) * 512],
                        in0=P[:, it, jb * 512 : (jb + 1) * 512],
                        in1=r128, op=ALU.mult,
                    )
                for jq in range(NT_J // JB):
                    jt = jb * (NT_J // JB) + jq
                    ps_H = ps_h.tile([128, 512], FP, tag="h", name=f"psH{b}_{jt}")
                    for ic in range(NT_I):
                        nc.tensor.matmul(
                            ps_H,
                            P[:, ic, jt * 128 : (jt + 1) * 128],
                            ctx_fr[:, ic, :],
                            start=(ic == 0),
                            stop=(ic == NT_I - 1),
                        )
                    h_sb = stage.tile([128, D], FP, tag="h_sb", name=f"hsb{b}_{jt}")
                    nc.scalar.copy(out=h_sb, in_=ps_H)
                    nc.sync.dma_start(
                        out=H_d[b, jt * 128 : (jt + 1) * 128, :], in_=h_sb
                    )

            # colsum over full rows (both halves folded)
            colsum = small.tile([128, NT_I], FP, tag="cs")
            for it in range(NT_I):
                nc.vector.reduce_sum(
                    out=colsum[:, it : it + 1], in_=P[:, it, :], axis=AX.X
                )

            # G right half = ctx * colsum
            for it in range(NT_I):
                gat = stage.tile([128, D], FP, tag="gat")
                nc.vector.tensor_scalar_mul(
                    gat, ctx_fr[:, it, :].bitcast(FP), colsum[:, it : it + 1]
                )
                nc.sync.dma_start(
                    out=G_d[b].rearrange("(t p) d2 -> p t d2", p=128)[:, it, D : 2 * D],
                    in_=gat,
                )

    _split_excess_waits(nc, mybir)
    _BUILT["nc"] = nc
    return nc


def kernel(context, query, w):
    from concourse.bass_utils import run_bass_kernel_spmd

    nc = _build()

    context = np.ascontiguousarray(context, dtype=np.float32)
    query = np.ascontiguousarray(query, dtype=np.float32)
    w = np.ascontiguousarray(w, dtype=np.float32)
    ident = np.eye(128, dtype=np.float32)
    ones = np.ones((128, 1), dtype=np.float32)

    in_maps = []
    for c in range(N_CORES):
        in_maps.append(
            {
                "context": context[c * B_LOC : (c + 1) * B_LOC],
                "query": query,
                "w": w,
                "ident": ident,
                "ones": ones,
            }
        )
    res = run_bass_kernel_spmd(nc, in_maps, list(range(N_CORES))).results
    G = np.concatenate([res[c]["G"] for c in range(N_CORES)], axis=0)
    H = np.concatenate([res[c]["H"] for c in range(N_CORES)], axis=0)
    return G, H


def kernel_traced(context, query, w):
    """Like kernel(), but with NTFF profiling; returns (G, H, BassKernelResults)."""
    from concourse.bass_utils import run_bass_kernel_spmd

    nc = _build()
    context = np.ascontiguousarray(context, dtype=np.float32)
    query = np.ascontiguousarray(query, dtype=np.float32)
    w = np.ascontiguousarray(w, dtype=np.float32)
    ident = np.eye(128, dtype=np.float32)
    ones = np.ones((128, 1), dtype=np.float32)
    in_maps = [
        {
            "context": context[c * B_LOC : (c + 1) * B_LOC],
            "query": query,
            "w": w,
            "ident": ident,
            "ones": ones,
        }
        for c in range(N_CORES)
    ]
    br = run_bass_kernel_spmd(nc, in_maps, list(range(N_CORES)), trace=True)
    res = br.results
    G = np.concatenate([res[c]["G"] for c in range(N_CORES)], axis=0)
    H = np.concatenate([res[c]["H"] for c in range(N_CORES)], axis=0)
    return G, H, br
